# revision 53
# baseline (speedup 1.0000x reference)
"""nn_InteractionLayer Bass/Tile kernel for 8 Trainium2 NeuronCores.

out = where(dist < 1, exp(-2*(1/dist - 1)^2), 0) @ (z @ W + B)

Row-parallel sharding per the problem's sharding hint: rows of dist_matrix
(and of the output) are split across the 8 cores; z/W/B are replicated.
Each core computes msg = z @ W + B (small GEMM, replicated) and its
[N/8, N] block of the masked sensitivity matrix, then a local
[N/8, N] x [N, D] matmul. No cross-device communication.

Device pipeline per core (per [128 j, 1024 i] tile of the host-transposed
dist block, streamed 64 times):
  DVE : u = reciprocal(r)                       (f32)
  POOL: m = (u > 1)                             (== dist < 1, exact; bf16)
  ACT : g = Derivative_Erf(sqrt2*u - sqrt2)     (= 2/sqrt(pi)*exp(-2(1/r-1)^2))
  DVE : w = g * m                               (bf16)
  PE  : outT[d, i] += msg[j, d] * w[j, i]       (bf16 matmul, fp32 PSUM)
The 2/sqrt(pi) factor of Derivative_Erf is folded into W/B on the host.
msg is computed on-device, interleaved with the stream: matmuls from
host-transposed z (zT, loaded as 16 chunk-DMAs woven into the dist stream)
with a ones-row trick to add B, PSUM->SBUF copies on ACT. Host does layout
only: transpose/slice/cast + constant folds. Output is written back in bf16
(out^T layout) and upcast/transposed on the host.

Walrus workaround: this container's neuronxcc/walrus build rejects any
instruction with more than one sync-wait ("Too many sync wait commands" /
"ISA wrong length"). Patch 1 rewrites the Tile kernel-tail drain into a
chain of single-wait NOPs; patch 2 post-processes the serialized BIR JSON,
splitting any remaining multi-wait instruction into single-wait same-engine
NoOps placed immediately before it (same happens-before edges).

"HW exec time" is reported from concourse's TimelineSim (the cost-model
timeline simulator), single-core — all 8 cores run the identical program.
"""

import json
import math

import numpy as np

N = 8192
D = 256
NCORES = 8
IPC = N // NCORES  # 1024 output rows per core
NJT = N // 128  # 64 j-tiles
SQRT2 = float(math.sqrt(2.0))
GAUSS_SCALE = float(math.sqrt(math.pi) / 2.0)  # undoes d_erf's 2/sqrt(pi)

_CACHE = {}


class _Result:
    def __init__(self, exec_time_ns):
        self.exec_time_ns = exec_time_ns


# ---------------------------------------------------------------------------
# walrus compatibility patches (see module docstring)
# ---------------------------------------------------------------------------

def _apply_walrus_patches():
    if _CACHE.get("patched"):
        return
    _CACHE["patched"] = True

    import concourse.mybir as mybir
    from concourse.bass import Bass
    from concourse.tile import TileContext
    from concourse.vector_clock import ScopedClock

    def _drain_and_barrier(self, tick_clock, wait_clock):
        nc = self.nc
        probe = nc.sync.nop(hint="tail_wait_probe", nofuse=True)
        wait_clock.add_sem_waits(
            probe.ins, ScopedClock({None: tick_clock.global_clock})
        )
        si = probe.ins.sync_info
        waits = list(si.on_wait) if si is not None else []
        # engine sems retire long before the final out-DMA completions, so
        # wait on DMA lanes last: earlier NOPs in the chain retire instantly
        waits.sort(key=lambda w: "DMA" in (getattr(w, "ant_name", "") or ""))
        first = waits[:1]
        probe.ins.sync_info = mybir.SyncInfo(on_wait=first, on_update=[])
        for w in waits[1:]:
            extra = nc.sync.nop(hint="tail_wait_chain", nofuse=True)
            extra.ins.sync_info = mybir.SyncInfo(on_wait=[w], on_update=[])
        nc.sync.drain()
        nc.all_engine_barrier()
        assert self.sems is not None
        popped = nc._tile_sem_poison_stack.pop()
        assert popped is self._sem_poison
        nc.clear_and_free_semaphores(list(self.sems.allocated().values()))
        nc.all_engine_barrier()

    TileContext._drain_and_barrier = _drain_and_barrier

    orig_to_json_bytes = Bass.to_json_bytes
    counter = [0]

    def to_json_bytes_split_waits(self) -> bytes:
        raw = orig_to_json_bytes(self)
        j = json.loads(raw)
        changed = False
        for fn in j.get("functions", []):
            for bb in fn.get("blocks", []):
                out = []
                for inst in bb.get("instructions", []):
                    si = inst.get("sync_info")
                    waits = si.get("on_wait") if si else None
                    if waits and len(waits) > 1:
                        changed = True
                        for w in waits[:-1]:
                            counter[0] += 1
                            out.append(
                                {
                                    "debug": inst.get("debug", 0),
                                    "engine": inst["engine"],
                                    "ins": [],
                                    "outs": [],
                                    "name": f"WSPLIT-{counter[0]}",
                                    "opcode": "NoOp",
                                    "sync_info": {
                                        "on_wait": [w],
                                        "on_update": [],
                                    },
                                }
                            )
                        si["on_wait"] = [waits[-1]]
                    out.append(inst)
                bb["instructions"] = out
        if not changed:
            return raw
        return json.dumps(j).encode()

    Bass.to_json_bytes = to_json_bytes_split_waits


# ---------------------------------------------------------------------------
# device kernel
# ---------------------------------------------------------------------------

def _build_nc():
    import concourse.bass as bass
    import concourse.mybir as mybir
    from concourse.alu_op_type import AluOpType
    from concourse.tile import TileContext

    f32 = mybir.dt.float32
    bf16 = mybir.dt.bfloat16
    AF = mybir.ActivationFunctionType

    nc = bass.Bass("TRN2", target_bir_lowering=False)

    # const AP for the d_erf bias (ACT float biases need a registered AP)
    cb = nc.alloc_sbuf_tensor("const-derf-bias", [128, 1], f32)
    nc.gpsimd.memset(cb.ap(), -SQRT2)
    nc.const_aps.aps[(f32, -SQRT2)] = cb.ap()
    nc.all_engine_barrier()

    dT = nc.dram_tensor("dT", [N, IPC], f32, kind="ExternalInput")
    zT = nc.dram_tensor("zT", [D, N], bf16, kind="ExternalInput")
    Wt = nc.dram_tensor("Wt", [D, D], bf16, kind="ExternalInput")
    Bt = nc.dram_tensor("Bt", [1, D], bf16, kind="ExternalInput")
    ones_d = nc.dram_tensor("ones", [1, N], bf16, kind="ExternalInput")
    outT = nc.dram_tensor("outT", [D, IPC], bf16, kind="ExternalOutput")

    R_PRE = 6  # r-tile DMA prefetch depth

    with TileContext(nc) as tc:
        with tc.tile_pool(name="persist", bufs=1) as persist, \
             tc.tile_pool(name="rpool", bufs=R_PRE + 2) as rpool, \
             tc.tile_pool(name="work", bufs=4) as work, \
             tc.tile_pool(name="outs", bufs=4) as outs, \
             tc.tile_pool(name="psum_msg", bufs=3, space="PSUM") as psum_msg, \
             tc.tile_pool(name="psum_acc", bufs=1, space="PSUM") as psum_acc:

            def issue_r(jt):
                r = rpool.tile([128, IPC], f32, tag="r", name=f"r{jt}")
                nc.sync.dma_start(out=r, in_=dT[jt * 128 : (jt + 1) * 128, :])
                return r

            # dist tiles lead the DMA queue so DVE starts ASAP
            r_tiles = [issue_r(jt) for jt in range(min(R_PRE, NJT))]

            # ---- small replicated operands ----
            w0 = persist.tile([128, D], bf16, tag="w0")
            nc.sync.dma_start(out=w0, in_=Wt[0:128, :])
            w1 = persist.tile([128, D], bf16, tag="w1")
            nc.sync.dma_start(out=w1, in_=Wt[128:256, :])
            bt = persist.tile([1, D], bf16, tag="bt")
            nc.sync.dma_start(out=bt, in_=Bt[0:1, :])
            ones = persist.tile([1, N], bf16, tag="ones")
            nc.sync.dma_start(out=ones, in_=ones_d[0:1, :])

            # zT in 16 chunk tiles, interleaved into the r stream below so
            # the 4 MiB load never stalls dist-tile delivery
            ZCH = N // 8  # 1024 columns per chunk
            zc = [
                persist.tile([128, ZCH], bf16, tag=f"zc{k}", name=f"zc{k}")
                for k in range(16)
            ]

            def issue_z(k):
                half, ch = k % 2, k // 2
                nc.sync.dma_start(
                    out=zc[k],
                    in_=zT[
                        half * 128 : (half + 1) * 128,
                        ch * ZCH : (ch + 1) * ZCH,
                    ],
                )

            # msg[j, d] in bf16, resident for the whole kernel
            msg = persist.tile([128, NJT, D], bf16, tag="msg")

            # out^T accumulators: [128 d, 512 i] x (2 d-chunks x 2 i-halves)
            accs = [
                psum_acc.tile([128, 512], f32, tag=f"acc{k}", name=f"acc{k}")
                for k in range(4)
            ]

            pipe = []

            def emit_mult(entry):
                g, m, jt = entry
                w = work.tile([128, IPC], bf16, tag="w", name=f"w{jt}")
                nc.vector.tensor_tensor(out=w, in0=g, in1=m, op=AluOpType.mult)
                start = jt == 0
                stop = jt == NJT - 1
                for dc in range(2):
                    lhsT = msg[:, jt, dc * 128 : (dc + 1) * 128]
                    for ih in range(2):
                        nc.tensor.matmul(
                            accs[dc * 2 + ih],
                            lhsT=lhsT,
                            rhs=w[:, ih * 512 : (ih + 1) * 512],
                            start=start,
                            stop=stop,
                        )

            for jt in range(NJT):
                js = slice(jt * 128, (jt + 1) * 128)
                r = r_tiles[jt]
                if jt + R_PRE < NJT:
                    r_tiles.append(issue_r(jt + R_PRE))
                # zT chunks woven into the dist stream, every other iteration
                # (chunk k is first needed by the msg matmul of tile 8*(k//2),
                # i.e. iteration 4k, so this is comfortably ahead); both
                # chunks of the first half-pair go first
                if jt == 0:
                    issue_z(0)
                    issue_z(1)
                elif jt % 3 == 0 and jt <= 42:
                    issue_z(jt // 3 + 1)

                # ---- msg tile: z @ W + B via ones-row trick ----
                ch, co = jt // 8, (jt % 8) * 128
                pm = psum_msg.tile([128, D], f32, tag="pm")
                nc.tensor.matmul(pm, lhsT=zc[2 * ch][:, co : co + 128],
                                 rhs=w0, start=True, stop=False)
                nc.tensor.matmul(pm, lhsT=zc[2 * ch + 1][:, co : co + 128],
                                 rhs=w1, start=False, stop=False)
                nc.tensor.matmul(pm, lhsT=ones[:, js], rhs=bt, start=False, stop=True)
                nc.scalar.copy(out=msg[:, jt, :], in_=pm)

                # ---- masked sensitivity of this j-tile ----
                u = work.tile([128, IPC], f32, tag="u")
                nc.vector.reciprocal(out=u, in_=r)
                g = work.tile([128, IPC], bf16, tag="g")
                nc.scalar.activation(
                    g, u, AF.Derivative_Erf, bias=-SQRT2, scale=SQRT2
                )
                # mask from r directly (exact, and independent of recip so
                # POOL runs right off the DMA); multiply split DVE 950 /
                # POOL 74 to balance both engines
                m = work.tile([128, IPC], bf16, tag="m")
                nc.gpsimd.tensor_scalar(
                    out=m, in0=r, scalar1=1.0, scalar2=None,
                    op0=AluOpType.is_lt,
                )
                emit_mult((g, m, jt))

            # ---- writeback (bf16; host upcasts); copies split across
            # ACT and DVE, which are both idle by now ----
            for dc in range(2):
                for ih in range(2):
                    ot = outs.tile([128, 512], bf16, tag="ot")
                    if ih == 0:
                        nc.scalar.copy(out=ot, in_=accs[dc * 2 + ih])
                    else:
                        nc.vector.tensor_copy(out=ot, in_=accs[dc * 2 + ih])
                    nc.sync.dma_start(
                        out=outT[
                            dc * 128 : (dc + 1) * 128,
                            ih * 512 : (ih + 1) * 512,
                        ],
                        in_=ot,
                    )
    return nc


def _get_nc():
    if "nc" not in _CACHE:
        _apply_walrus_patches()
        _CACHE["nc"] = _build_nc()
    return _CACHE["nc"]


def _sim_exec_time_ns():
    if "sim_ns" not in _CACHE:
        from concourse.timeline_sim import TimelineSim

        tl = TimelineSim(_get_nc())
        _CACHE["sim_ns"] = int(tl.simulate())
    return _CACHE["sim_ns"]


# ---------------------------------------------------------------------------
# host entry point
# ---------------------------------------------------------------------------

def kernel(z, dist_matrix, W, B, _trace=True):
    import ml_dtypes

    from concourse.bass_utils import run_bass_kernel_spmd

    bf16 = ml_dtypes.bfloat16

    z = np.asarray(z, np.float32)
    dist = np.asarray(dist_matrix, np.float32)
    W = np.asarray(W, np.float32)
    B = np.asarray(B, np.float32)

    # host-side layout + constant folds (math happens on device)
    zTb = np.ascontiguousarray(z.T).astype(bf16)  # [D, N]
    Wb = (W * GAUSS_SCALE).astype(bf16)  # [D, D]
    Bb = (B * GAUSS_SCALE).astype(bf16).reshape(1, D)
    ones = np.ones((1, N), bf16)
    distT = dist.T  # view
    in_maps = []
    for c in range(NCORES):
        dT = np.ascontiguousarray(distT[:, c * IPC : (c + 1) * IPC])
        in_maps.append(
            {"dT": dT, "zT": zTb, "Wt": Wb, "Bt": Bb, "ones": ones}
        )

    nc = _get_nc()
    res = run_bass_kernel_spmd(
        nc, in_maps, core_ids=list(range(NCORES)), trace=False
    )

    out = np.empty((N, D), np.float32)
    for c in range(NCORES):
        out[c * IPC : (c + 1) * IPC, :] = (
            res.results[c]["outT"].astype(np.float32).T
        )

    exec_ns = _sim_exec_time_ns() if _trace else None
    _CACHE["last"] = _Result(exec_ns)
    return out
